# revision 1
# baseline (speedup 1.0000x reference)
"""GNN message passing + 3x conv3x3 + leaky-relu, distributed over 8 trn2 NeuronCores.

Strategy (node-sharded, 128 nodes/core):
- Pooling (pos/neg masked bidirectional scatter-add) is done entirely by SWDGE
  indirect-gather DMAs with compute_op=add: the feats table is laid out as
  (node, channel) rows of 4 KiB; each gather round pulls 128 rows (4 nodes x
  2 signs x 16 ch) and accumulates into an SBUF tile that is ALREADY in
  conv layout (partition=channel). Zero compute-engine cost, exact fp32.
- Convs run as 9 shifted-tap matmuls over a 34x34 zero-padded grid using
  strided access-pattern views (no im2col), bf16 operands, fp32 PSUM
  accumulation. Four nodes are packed per 128-partition bundle and mapped to
  disjoint 32x32 PE sub-array rectangles via tile_position, so the 128x128
  array stays busy despite 32-channel convs.
- leaky_relu(x) = x + relu(-0.9 x): one ScalarE activation + one VectorE
  tensor_tensor add per bundle, full 128-lane ops.
"""

import numpy as np

N, C, H, W = 1024, 16, 32, 32
NCORES = 8
NPC = N // NCORES            # nodes per core
GROUPS = NPC // 4            # acc groups of 4 nodes per core
CONV_ROUNDS = NPC // 16      # 16 nodes per conv round (4 bundles)
HP = WP = H + 2
GRID = HP * WP
HW = H * W
ZROW = 16 * N                # first zero row in the gather table

_prog_cache = {}


def _make_tile_context(nc):
    """TileContext whose lowering splits multi-sem waits onto nop carriers
    (this walrus build accepts at most one sync wait per instruction) and
    whose tail drain does the same."""
    import concourse.mybir as mybir
    import concourse.tile as tile

    class _TC(tile.TileContext):
        def _lower_ordered_insts(self, ordered):
            for bb_name, insts in ordered.items():
                out = []
                for inst in insts:
                    si = inst.sync_info
                    waits = list(si.on_wait) if si is not None and si.on_wait else []
                    if len(waits) > 1:
                        for w in waits[:-1]:
                            car = mybir.InstNoOp(
                                name=self.nc.get_next_instruction_name(),
                                ins=[], outs=[])
                            car.engine = inst.engine
                            car.sync_info = mybir.SyncInfo(on_wait=[w], on_update=[])
                            self.nc.register_instruction(car, overwrite=True)
                            out.append(car)
                        inst.sync_info = mybir.SyncInfo(
                            on_wait=[waits[-1]],
                            on_update=list(si.on_update) if si.on_update else [])
                    out.append(inst)
                insts[:] = out
            return super()._lower_ordered_insts(ordered)

        def _drain_and_barrier(self, tick_clock, wait_clock):
            clock = tick_clock.global_clock
            allocated = wait_clock.sems.allocated()
            for proc, tick in enumerate(clock):
                if tick > 0 and proc in allocated:
                    n = self.nc.sync.nop(nofuse=True, hint="tailwait")
                    n.wait_op(allocated[proc], tick, "sem-ge")
            self.nc.sync.drain()
            self.nc.all_engine_barrier()
            assert self.sems is not None
            popped = self.nc._tile_sem_poison_stack.pop()
            assert popped is self._sem_poison
            self.nc.clear_and_free_semaphores(list(self.sems.allocated().values()))
            self.nc.all_engine_barrier()

    return _TC(nc)


def _build_program(r_list, r_off, r_total, with_bias, variant="full"):
    import os
    import concourse.bass as bass
    import concourse.mybir as mybir

    reps = 1
    if "x" in variant:
        variant, _, r = variant.partition("x")
        reps = int(r)
    do_gather = variant in ("full", "gather")
    do_conv = variant in ("full", "conv")

    f32 = mybir.dt.float32
    bf16 = mybir.dt.bfloat16
    i32 = mybir.dt.int32
    AF = mybir.ActivationFunctionType
    ALU = mybir.AluOpType

    nc = bass.Bass()
    tab_d = nc.dram_tensor("tab", [16 * N + 16, HW], f32, kind="ExternalInput")
    gidx_d = nc.dram_tensor("gidx", [128, r_total], i32, kind="ExternalInput")
    fown_d = nc.dram_tensor("fown", [NPC * C, HW], f32, kind="ExternalInput")
    w1pn_d = nc.dram_tensor("w1pn", [128, 9 * 32], bf16, kind="ExternalInput")
    w1s_d = nc.dram_tensor("w1s", [128, 9 * 32], bf16, kind="ExternalInput")
    w2_d = nc.dram_tensor("w2", [128, 9 * 32], bf16, kind="ExternalInput")
    w3_d = nc.dram_tensor("w3", [128, 9 * 16], bf16, kind="ExternalInput")
    bias_d = nc.dram_tensor("bias", [128, 3], f32, kind="ExternalInput")
    y_d = nc.dram_tensor("y", [NPC * C, HW], f32, kind="ExternalOutput")

    def valid(ap_grid):
        # [p, GRID] tile AP -> [p, 32, 32] interior view of the 34x34 grid
        return ap_grid.rearrange("p (h w) -> p h w", w=WP)[:, 1:H + 1, 1:W + 1]

    def tap_view(ap_grid, base, k, dy, dx, h0):
        # rhs view for tap (dy,dx), output rows [h0, h0+16), K channels at
        # partition `base`
        g3 = ap_grid.rearrange("p (h w) -> p h w", w=WP)
        return g3[base:base + k, h0 + dy:h0 + dy + 16, dx:dx + W]

    tc = _make_tile_context(nc)
    with tc:
        with (tc.tile_pool(name="cw", bufs=1) as cw,
              tc.tile_pool(name="accp", bufs=6) as accp,
              tc.tile_pool(name="x1pnp", bufs=3) as x1pnp,
              tc.tile_pool(name="fop", bufs=3) as fop,
              tc.tile_pool(name="x1sp", bufs=3) as x1sp,
              tc.tile_pool(name="x2p", bufs=3) as x2p,
              tc.tile_pool(name="x3p", bufs=3) as x3p,
              tc.tile_pool(name="r2p", bufs=3) as r2p,
              tc.tile_pool(name="osbp", bufs=3) as osbp,
              tc.tile_pool(name="psp", bufs=4, space="PSUM") as psp):
            idx_t = cw.tile([128, r_total], i32)
            nc.sync.dma_start(out=idx_t[:], in_=gidx_d[:])
            w1pn_t = cw.tile([128, 9 * 32], bf16)
            nc.sync.dma_start(out=w1pn_t[:], in_=w1pn_d[:])
            w1s_t = cw.tile([128, 9 * 32], bf16)
            nc.sync.dma_start(out=w1s_t[:], in_=w1s_d[:])
            w2_t = cw.tile([128, 9 * 32], bf16)
            nc.sync.dma_start(out=w2_t[:], in_=w2_d[:])
            w3_t = cw.tile([128, 9 * 16], bf16)
            nc.sync.dma_start(out=w3_t[:], in_=w3_d[:])
            if with_bias:
                bias_t = cw.tile([128, 3], f32)
                nc.sync.dma_start(out=bias_t[:], in_=bias_d[:])

            memset_count = {}

            def fresh_grid(pool, name):
                t = pool.tile([128, GRID], bf16, tag=name)
                c = memset_count.get(name, 0)
                if c < 3:  # pool bufs
                    nc.vector.memset(t[:], 0.0)
                    memset_count[name] = c + 1
                return t

            def fresh_fo(pool, name):
                t = pool.tile([128, HW], bf16, tag=name)
                c = memset_count.get(name, 0)
                if c < 3:
                    nc.vector.memset(t[:], 0.0)
                    memset_count[name] = c + 1
                return t

            for rnd in [r for _ in range(reps) for r in range(CONV_ROUNDS)]:
                x1pn_tiles = []
                x1s_tiles = []
                for b in range(4):
                    g = 4 * rnd + b
                    # ---- pooling: accumulate gathers into acc (f32, exact)
                    acc_t = accp.tile([128, HW], f32, tag="acc")
                    if do_gather:
                        for r in range(r_list[g]):
                            col = r_off[g] + r
                            nc.gpsimd.indirect_dma_start(
                                out=acc_t[:], out_offset=None, in_=tab_d[:],
                                in_offset=bass.IndirectOffsetOnAxis(
                                    ap=idx_t[:, col:col + 1], axis=0),
                                compute_op=ALU.bypass if r == 0 else ALU.add)
                    else:
                        nc.vector.memset(acc_t[:], 0.0)
                    if not do_conv:
                        continue
                    # ---- X1 pos/neg grid (bf16, strided valid write)
                    x1 = fresh_grid(x1pnp, "x1pn")
                    nc.vector.tensor_copy(
                        out=valid(x1[:]),
                        in_=acc_t[:].rearrange("p (h w) -> p h w", w=W))
                    x1pn_tiles.append(x1)
                    # ---- X1 self grid: stage feats_own (cast bf16) then copy
                    fo = fresh_fo(fop, "fo")
                    for j in range(4):
                        slot = 16 * rnd + 4 * b + j
                        nc.gpsimd.dma_start(
                            out=fo[32 * j:32 * j + C, :],
                            in_=fown_d[C * slot:C * slot + C, :])
                    x1s = fresh_grid(x1sp, "x1s")
                    nc.vector.tensor_copy(
                        out=valid(x1s[:]),
                        in_=fo[:].rearrange("p (h w) -> p h w", w=W))
                    x1s_tiles.append(x1s)

                for b in range(4 if do_conv else 0):
                    x1, x1s = x1pn_tiles[b], x1s_tiles[b]
                    # ---- conv1: pass1 K=32 (pos+neg), pass2 K=16 (self)
                    ps1 = psp.tile([128, HW], f32, tag="ps")
                    ps1v = ps1[:].rearrange("p (h w) -> p h w", w=W)
                    for j in range(4):
                        cs = (j + b) % 4
                        for h0 in (0, 16):
                            for t in range(9):
                                dy, dx = t // 3, t % 3
                                nc.tensor.matmul(
                                    out=ps1v[32 * cs:32 * cs + 32, h0:h0 + 16, :],
                                    lhsT=w1pn_t[32 * j:32 * j + 32, t * 32:t * 32 + 32],
                                    rhs=tap_view(x1[:], 32 * j, 32, dy, dx, h0),
                                    start=(t == 0), stop=False,
                                    tile_position=(32 * j, 32 * cs))
                            for t in range(9):
                                dy, dx = t // 3, t % 3
                                nc.tensor.matmul(
                                    out=ps1v[32 * cs:32 * cs + 32, h0:h0 + 16, :],
                                    lhsT=w1s_t[32 * j:32 * j + 32, t * 32:t * 32 + 32],
                                    rhs=tap_view(x1s[:], 32 * j, 32, dy, dx, h0),
                                    start=False, stop=(t == 8),
                                    tile_position=(32 * j, 32 * cs))
                    r2a = r2p.tile([128, HW], bf16, tag="r2")
                    nc.scalar.activation(out=r2a[:], in_=ps1[:], func=AF.Relu,
                                         scale=-0.9)
                    x2 = fresh_grid(x2p, "x2")
                    nc.vector.tensor_tensor(
                        out=valid(x2[:]),
                        in0=ps1[:].rearrange("p (h w) -> p h w", w=W),
                        in1=r2a[:].rearrange("p (h w) -> p h w", w=W),
                        op=ALU.add)

                    # ---- conv2 (K=32)
                    ps2 = psp.tile([128, HW], f32, tag="ps")
                    ps2v = ps2[:].rearrange("p (h w) -> p h w", w=W)
                    for q in range(4):
                        cs = (q + b + 1) % 4
                        for h0 in (0, 16):
                            for t in range(9):
                                dy, dx = t // 3, t % 3
                                nc.tensor.matmul(
                                    out=ps2v[32 * cs:32 * cs + 32, h0:h0 + 16, :],
                                    lhsT=w2_t[32 * q:32 * q + 32, t * 32:t * 32 + 32],
                                    rhs=tap_view(x2[:], 32 * q, 32, dy, dx, h0),
                                    start=(t == 0), stop=(t == 8),
                                    tile_position=(32 * q, 32 * cs))
                    r2b = r2p.tile([128, HW], bf16, tag="r2")
                    nc.scalar.activation(out=r2b[:], in_=ps2[:], func=AF.Relu,
                                         scale=-0.9)
                    x3 = fresh_grid(x3p, "x3")
                    nc.vector.tensor_tensor(
                        out=valid(x3[:]),
                        in0=ps2[:].rearrange("p (h w) -> p h w", w=W),
                        in1=r2b[:].rearrange("p (h w) -> p h w", w=W),
                        op=ALU.add)

                    # ---- conv3 (K=32, M=16)
                    ps3 = psp.tile([128, HW], f32, tag="ps")
                    ps3v = ps3[:].rearrange("p (h w) -> p h w", w=W)
                    for q in range(4):
                        cs = (q + b + 2) % 4
                        for h0 in (0, 16):
                            for t in range(9):
                                dy, dx = t // 3, t % 3
                                nc.tensor.matmul(
                                    out=ps3v[32 * cs:32 * cs + 16, h0:h0 + 16, :],
                                    lhsT=w3_t[32 * q:32 * q + 32, t * 16:t * 16 + 16],
                                    rhs=tap_view(x3[:], 32 * q, 32, dy, dx, h0),
                                    start=(t == 0), stop=(t == 8),
                                    tile_position=(32 * q, 32 * cs))
                    r2c = r2p.tile([128, HW], bf16, tag="r2")
                    nc.scalar.activation(out=r2c[:], in_=ps3[:], func=AF.Relu,
                                         scale=-0.9)
                    osb = osbp.tile([128, HW], f32, tag="osb")
                    nc.vector.tensor_tensor(out=osb[:], in0=ps3[:], in1=r2c[:],
                                            op=ALU.add)
                    for j in range(4):
                        q1 = (j + b) % 4
                        q2 = (q1 + b + 1) % 4
                        q3 = (q2 + b + 2) % 4
                        slot = 16 * rnd + 4 * b + j
                        nc.sync.dma_start(
                            out=y_d[C * slot:C * slot + C, :],
                            in_=osb[32 * q3:32 * q3 + C, :])
    return nc


def _host_prep(feats, edges, w1, b1, w2, b2, w3, b3):
    import ml_dtypes

    feats = np.ascontiguousarray(np.asarray(feats, dtype=np.float32))
    edges = np.asarray(edges)
    w1 = np.asarray(w1, dtype=np.float32)
    w2 = np.asarray(w2, dtype=np.float32)
    w3 = np.asarray(w3, dtype=np.float32)

    # per-(node, sign) contribution lists
    contrib = [([], []) for _ in range(N)]
    for s, sg, d in edges.tolist():
        si = 0 if sg > 0 else 1
        contrib[d][si].append(s)
        contrib[s][si].append(d)

    # per-core slot ordering: sort by max degree so groups of 4 have similar
    # round counts (minimises padded gather rounds)
    slot2node = []
    for k in range(NCORES):
        nodes = list(range(NPC * k, NPC * (k + 1)))
        nodes.sort(key=lambda n: -max(len(contrib[n][0]), len(contrib[n][1])))
        slot2node.append(nodes)

    # group round counts, maxed across cores (program must be SPMD-uniform)
    r_list = []
    for g in range(GROUPS):
        r = 1
        for k in range(NCORES):
            for j in range(4):
                n = slot2node[k][4 * g + j]
                r = max(r, len(contrib[n][0]), len(contrib[n][1]))
        r_list.append(r)
    r_off = np.concatenate([[0], np.cumsum(r_list)]).astype(int)
    r_total = int(r_off[-1])

    feats2d = feats.reshape(N * C, HW)
    tab = np.concatenate([feats2d, np.zeros((C, HW), np.float32)], axis=0)

    # weight tiles (lhsT layout, replicated across the 4 row slots)
    def wtile(w, ci_lo, ci_n, co_n):
        t = np.zeros((128, 9 * co_n), np.float32)
        for rs in range(4):
            for tp in range(9):
                dy, dx = tp // 3, tp % 3
                t[32 * rs:32 * rs + ci_n, tp * co_n:(tp + 1) * co_n] = \
                    w[:, ci_lo:ci_lo + ci_n, dy, dx].T
        return t.astype(ml_dtypes.bfloat16)

    w1pn = wtile(w1, C, 2 * C, 2 * C)
    w1s = wtile(w1, 0, C, 2 * C)
    w2t = wtile(w2, 0, 2 * C, 2 * C)
    w3t = wtile(w3, 0, 2 * C, C)
    biases = np.zeros((128, 3), np.float32)

    in_maps = []
    chan = np.arange(128) % C
    for k in range(NCORES):
        gidx = np.empty((128, r_total), np.int32)
        gidx[:] = (ZROW + chan)[:, None]
        for g in range(GROUPS):
            for j in range(4):
                n = slot2node[k][4 * g + j]
                for si in range(2):
                    lst = contrib[n][si]
                    base = 32 * j + 16 * si
                    for r, m in enumerate(lst):
                        gidx[base:base + C, r_off[g] + r] = C * m + chan[:C]
        rows = np.concatenate(
            [np.arange(C * n, C * n + C) for n in slot2node[k]])
        fown = feats2d[rows]
        in_maps.append({
            "tab": tab, "gidx": gidx, "fown": np.ascontiguousarray(fown),
            "w1pn": w1pn, "w1s": w1s, "w2": w2t, "w3": w3t, "bias": biases,
        })
    return in_maps, slot2node, tuple(r_list), tuple(r_off[:-1].tolist()), r_total


def kernel(feats, edges, w1, b1, w2, b2, w3, b3):
    from concourse.bass_utils import run_bass_kernel_spmd

    in_maps, slot2node, r_list, r_off, r_total = _host_prep(
        feats, edges, w1, b1, w2, b2, w3, b3)
    with_bias = bool(np.any(np.asarray(b1)) or np.any(np.asarray(b2))
                     or np.any(np.asarray(b3)))
    assert not with_bias, "nonzero conv biases not implemented"

    key = (r_list, with_bias)
    nc = _prog_cache.get(key)
    if nc is None:
        nc = _build_program(r_list, r_off, r_total, with_bias)
        _prog_cache[key] = nc

    import os
    trace = bool(os.environ.get("KERNEL_TRACE"))
    res = run_bass_kernel_spmd(nc, in_maps, core_ids=list(range(NCORES)),
                               trace=trace)
    if trace:
        global last_results
        last_results = res

    out = np.empty((N, C, H, W), np.float32)
    for k in range(NCORES):
        yk = res.results[k]["y"]
        for i, n in enumerate(slot2node[k]):
            out[n] = yk[C * i:C * i + C].reshape(C, H, W)
    return out



# revision 4
# speedup vs baseline: 3.2711x; 3.2711x over previous
"""GNN message passing + 3x conv3x3 + leaky-relu, distributed over 8 trn2 NeuronCores.

Strategy (node-sharded, 128 nodes/core):
- Pooling (pos/neg masked bidirectional scatter-add) is done entirely by SWDGE
  indirect-gather DMAs with compute_op=add: the feats table is laid out as
  (node, channel) rows of 4 KiB; each gather round pulls 128 rows (4 nodes x
  2 signs x 16 ch) and accumulates into an SBUF tile that is ALREADY in
  conv layout (partition=channel). Zero compute-engine cost, exact fp32.
- Convs run as 9 shifted-tap matmuls over a 34x34 zero-padded grid using
  strided access-pattern views (no im2col), bf16 operands, fp32 PSUM
  accumulation. Four nodes are packed per 128-partition bundle and mapped to
  disjoint 32x32 PE sub-array rectangles via tile_position, so the 128x128
  array stays busy despite 32-channel convs.
- leaky_relu(x) = x + relu(-0.9 x): one ScalarE activation + one VectorE
  tensor_tensor add per bundle, full 128-lane ops.
"""

import numpy as np

N, C, H, W = 1024, 16, 32, 32
NCORES = 8
NPC = N // NCORES            # nodes per core
GROUPS = NPC // 4            # acc groups of 4 nodes per core
CONV_ROUNDS = NPC // 16      # 16 nodes per conv round (4 bundles)
HP = WP = H + 2
GRID = HP * WP
HW = H * W
ZROW = 16 * N                # first zero row in the gather table

_prog_cache = {}


def _make_tile_context(nc):
    """TileContext whose lowering splits multi-sem waits onto nop carriers
    (this walrus build accepts at most one sync wait per instruction) and
    whose tail drain does the same."""
    import concourse.mybir as mybir
    import concourse.tile as tile

    class _TC(tile.TileContext):
        def _lower_ordered_insts(self, ordered):
            for bb_name, insts in ordered.items():
                out = []
                for inst in insts:
                    si = inst.sync_info
                    waits = list(si.on_wait) if si is not None and si.on_wait else []
                    if len(waits) > 1:
                        for w in waits[:-1]:
                            car = mybir.InstNoOp(
                                name=self.nc.get_next_instruction_name(),
                                ins=[], outs=[])
                            car.engine = inst.engine
                            car.sync_info = mybir.SyncInfo(on_wait=[w], on_update=[])
                            self.nc.register_instruction(car, overwrite=True)
                            out.append(car)
                        inst.sync_info = mybir.SyncInfo(
                            on_wait=[waits[-1]],
                            on_update=list(si.on_update) if si.on_update else [])
                    out.append(inst)
                insts[:] = out
            return super()._lower_ordered_insts(ordered)

        def _drain_and_barrier(self, tick_clock, wait_clock):
            clock = tick_clock.global_clock
            allocated = wait_clock.sems.allocated()
            for proc, tick in enumerate(clock):
                if tick > 0 and proc in allocated:
                    n = self.nc.sync.nop(nofuse=True, hint="tailwait")
                    n.wait_op(allocated[proc], tick, "sem-ge")
            self.nc.sync.drain()
            self.nc.all_engine_barrier()
            assert self.sems is not None
            popped = self.nc._tile_sem_poison_stack.pop()
            assert popped is self._sem_poison
            self.nc.clear_and_free_semaphores(list(self.sems.allocated().values()))
            self.nc.all_engine_barrier()

    return _TC(nc)


def _build_program(r_list, r_off, r_total, with_bias, variant="full"):
    import os
    import concourse.bass as bass
    import concourse.mybir as mybir

    reps = 1
    if "x" in variant:
        variant, _, r = variant.partition("x")
        reps = int(r)
    do_gather = variant in ("full", "gather")
    do_conv = variant in ("full", "conv")

    f32 = mybir.dt.float32
    bf16 = mybir.dt.bfloat16
    i32 = mybir.dt.int32
    AF = mybir.ActivationFunctionType
    ALU = mybir.AluOpType

    nc = bass.Bass()
    tab_d = nc.dram_tensor("tab", [16 * N + 16, HW], f32, kind="ExternalInput")
    gidx_d = nc.dram_tensor("gidx", [128, r_total], i32, kind="ExternalInput")
    fown_d = nc.dram_tensor("fown", [NPC * C, HW], f32, kind="ExternalInput")
    w1pn_d = nc.dram_tensor("w1pn", [128, 9 * 32], bf16, kind="ExternalInput")
    w1s_d = nc.dram_tensor("w1s", [128, 9 * 32], bf16, kind="ExternalInput")
    w2_d = nc.dram_tensor("w2", [128, 9 * 32], bf16, kind="ExternalInput")
    w3_d = nc.dram_tensor("w3", [128, 9 * 16], bf16, kind="ExternalInput")
    bias_d = nc.dram_tensor("bias", [128, 3], f32, kind="ExternalInput")
    y_d = nc.dram_tensor("y", [NPC * C, HW], f32, kind="ExternalOutput")

    def valid(ap_grid):
        # [p, GRID] tile AP -> [p, 32, 32] interior view of the 34x34 grid
        return ap_grid.rearrange("p (h w) -> p h w", w=WP)[:, 1:H + 1, 1:W + 1]

    def tap_view(ap_grid, base, k, dy, dx, h0):
        # rhs view for tap (dy,dx), output rows [h0, h0+16), K channels at
        # partition `base`
        g3 = ap_grid.rearrange("p (h w) -> p h w", w=WP)
        return g3[base:base + k, h0 + dy:h0 + dy + 16, dx:dx + W]

    tc = _make_tile_context(nc)
    with tc:
        with (tc.tile_pool(name="cw", bufs=1) as cw,
              tc.tile_pool(name="accp", bufs=6) as accp,
              tc.tile_pool(name="x1pnp", bufs=3) as x1pnp,
              tc.tile_pool(name="fop", bufs=3) as fop,
              tc.tile_pool(name="x1sp", bufs=3) as x1sp,
              tc.tile_pool(name="x2p", bufs=3) as x2p,
              tc.tile_pool(name="x3p", bufs=3) as x3p,
              tc.tile_pool(name="r2p", bufs=3) as r2p,
              tc.tile_pool(name="osbp", bufs=3) as osbp,
              tc.tile_pool(name="psp", bufs=4, space="PSUM") as psp):
            idx_t = cw.tile([128, r_total], i32)
            nc.sync.dma_start(out=idx_t[:], in_=gidx_d[:])
            w1pn_t = cw.tile([128, 9 * 32], bf16)
            nc.sync.dma_start(out=w1pn_t[:], in_=w1pn_d[:])
            w1s_t = cw.tile([128, 9 * 32], bf16)
            nc.sync.dma_start(out=w1s_t[:], in_=w1s_d[:])
            w2_t = cw.tile([128, 9 * 32], bf16)
            nc.sync.dma_start(out=w2_t[:], in_=w2_d[:])
            w3_t = cw.tile([128, 9 * 16], bf16)
            nc.sync.dma_start(out=w3_t[:], in_=w3_d[:])
            if with_bias:
                bias_t = cw.tile([128, 3], f32)
                nc.sync.dma_start(out=bias_t[:], in_=bias_d[:])

            memset_count = {}

            def fresh_grid(pool, name):
                t = pool.tile([128, GRID], bf16, tag=name)
                c = memset_count.get(name, 0)
                if c < 3:  # pool bufs
                    nc.vector.memset(t[:], 0.0)
                    memset_count[name] = c + 1
                return t

            def fresh_fo(pool, name):
                t = pool.tile([128, HW], bf16, tag=name)
                c = memset_count.get(name, 0)
                if c < 3:
                    nc.vector.memset(t[:], 0.0)
                    memset_count[name] = c + 1
                return t

            for rnd in [r for _ in range(reps) for r in range(CONV_ROUNDS)]:
                x1pn_tiles = []
                x1s_tiles = []
                for b in range(4):
                    g = 4 * rnd + b
                    # ---- pooling: accumulate gathers into acc (f32, exact)
                    acc_t = accp.tile([128, HW], f32, tag="acc")
                    if do_gather:
                        for r in range(r_list[g]):
                            col = r_off[g] + r
                            nc.gpsimd.indirect_dma_start(
                                out=acc_t[:], out_offset=None, in_=tab_d[:],
                                in_offset=bass.IndirectOffsetOnAxis(
                                    ap=idx_t[:, col:col + 1], axis=0),
                                compute_op=ALU.bypass if r == 0 else ALU.add)
                    else:
                        nc.vector.memset(acc_t[:], 0.0)
                    if not do_conv:
                        continue
                    # ---- X1 pos/neg grid (bf16, strided valid write)
                    x1 = fresh_grid(x1pnp, "x1pn")
                    nc.vector.tensor_copy(
                        out=valid(x1[:]),
                        in_=acc_t[:].rearrange("p (h w) -> p h w", w=W))
                    x1pn_tiles.append(x1)
                    # ---- X1 self grid: stage feats_own (cast bf16) then copy
                    fo = fresh_fo(fop, "fo")
                    for j in range(4):
                        slot = 16 * rnd + 4 * b + j
                        nc.gpsimd.dma_start(
                            out=fo[32 * j:32 * j + C, :],
                            in_=fown_d[C * slot:C * slot + C, :])
                    x1s = fresh_grid(x1sp, "x1s")
                    nc.vector.tensor_copy(
                        out=valid(x1s[:]),
                        in_=fo[:].rearrange("p (h w) -> p h w", w=W))
                    x1s_tiles.append(x1s)

                for b in range(4 if do_conv else 0):
                    x1, x1s = x1pn_tiles[b], x1s_tiles[b]
                    # ---- conv1: pass1 K=32 (pos+neg), pass2 K=16 (self)
                    ps1 = psp.tile([128, HW], f32, tag="ps")
                    ps1v = ps1[:].rearrange("p (h w) -> p h w", w=W)
                    for h0 in (0, 16):
                        for t in range(9):
                            dy, dx = t // 3, t % 3
                            for j in range(4):
                                cs = (j + b) % 4
                                nc.tensor.matmul(
                                    out=ps1v[32 * cs:32 * cs + 32, h0:h0 + 16, :],
                                    lhsT=w1pn_t[32 * j:32 * j + 32, t * 32:t * 32 + 32],
                                    rhs=tap_view(x1[:], 32 * j, 32, dy, dx, h0),
                                    start=(t == 0), stop=False,
                                    tile_position=(32 * j, 32 * cs))
                        for t in range(9):
                            dy, dx = t // 3, t % 3
                            for j in range(4):
                                cs = (j + b) % 4
                                nc.tensor.matmul(
                                    out=ps1v[32 * cs:32 * cs + 32, h0:h0 + 16, :],
                                    lhsT=w1s_t[32 * j:32 * j + 32, t * 32:t * 32 + 32],
                                    rhs=tap_view(x1s[:], 32 * j, 32, dy, dx, h0),
                                    start=False, stop=(t == 8),
                                    tile_position=(32 * j, 32 * cs))
                    r2a = r2p.tile([128, HW], bf16, tag="r2")
                    nc.scalar.activation(out=r2a[:], in_=ps1[:], func=AF.Relu,
                                         scale=-0.9)
                    x2 = fresh_grid(x2p, "x2")
                    nc.vector.tensor_tensor(
                        out=valid(x2[:]),
                        in0=ps1[:].rearrange("p (h w) -> p h w", w=W),
                        in1=r2a[:].rearrange("p (h w) -> p h w", w=W),
                        op=ALU.add)

                    # ---- conv2 (K=32)
                    ps2 = psp.tile([128, HW], f32, tag="ps")
                    ps2v = ps2[:].rearrange("p (h w) -> p h w", w=W)
                    for h0 in (0, 16):
                        for t in range(9):
                            dy, dx = t // 3, t % 3
                            for q in range(4):
                                cs = (q + b + 1) % 4
                                nc.tensor.matmul(
                                    out=ps2v[32 * cs:32 * cs + 32, h0:h0 + 16, :],
                                    lhsT=w2_t[32 * q:32 * q + 32, t * 32:t * 32 + 32],
                                    rhs=tap_view(x2[:], 32 * q, 32, dy, dx, h0),
                                    start=(t == 0), stop=(t == 8),
                                    tile_position=(32 * q, 32 * cs))
                    r2b = r2p.tile([128, HW], bf16, tag="r2")
                    nc.scalar.activation(out=r2b[:], in_=ps2[:], func=AF.Relu,
                                         scale=-0.9)
                    x3 = fresh_grid(x3p, "x3")
                    nc.vector.tensor_tensor(
                        out=valid(x3[:]),
                        in0=ps2[:].rearrange("p (h w) -> p h w", w=W),
                        in1=r2b[:].rearrange("p (h w) -> p h w", w=W),
                        op=ALU.add)

                    # ---- conv3 (K=32, M=16)
                    ps3 = psp.tile([128, HW], f32, tag="ps")
                    ps3v = ps3[:].rearrange("p (h w) -> p h w", w=W)
                    for h0 in (0, 16):
                        for t in range(9):
                            dy, dx = t // 3, t % 3
                            for q in range(4):
                                cs = (q + b + 2) % 4
                                nc.tensor.matmul(
                                    out=ps3v[32 * cs:32 * cs + 16, h0:h0 + 16, :],
                                    lhsT=w3_t[32 * q:32 * q + 32, t * 16:t * 16 + 16],
                                    rhs=tap_view(x3[:], 32 * q, 32, dy, dx, h0),
                                    start=(t == 0), stop=(t == 8),
                                    tile_position=(32 * q, 32 * cs))
                    r2c = r2p.tile([128, HW], bf16, tag="r2")
                    nc.scalar.activation(out=r2c[:], in_=ps3[:], func=AF.Relu,
                                         scale=-0.9)
                    osb = osbp.tile([128, HW], f32, tag="osb")
                    nc.vector.tensor_tensor(out=osb[:], in0=ps3[:], in1=r2c[:],
                                            op=ALU.add)
                    for j in range(4):
                        q1 = (j + b) % 4
                        q2 = (q1 + b + 1) % 4
                        q3 = (q2 + b + 2) % 4
                        slot = 16 * rnd + 4 * b + j
                        nc.sync.dma_start(
                            out=y_d[C * slot:C * slot + C, :],
                            in_=osb[32 * q3:32 * q3 + C, :])
    return nc


def _host_prep(feats, edges, w1, b1, w2, b2, w3, b3):
    import ml_dtypes

    feats = np.ascontiguousarray(np.asarray(feats, dtype=np.float32))
    edges = np.asarray(edges)
    w1 = np.asarray(w1, dtype=np.float32)
    w2 = np.asarray(w2, dtype=np.float32)
    w3 = np.asarray(w3, dtype=np.float32)

    # per-(node, sign) contribution lists
    contrib = [([], []) for _ in range(N)]
    for s, sg, d in edges.tolist():
        si = 0 if sg > 0 else 1
        contrib[d][si].append(s)
        contrib[s][si].append(d)

    # per-core slot ordering: sort by max degree so groups of 4 have similar
    # round counts (minimises padded gather rounds)
    slot2node = []
    for k in range(NCORES):
        nodes = list(range(NPC * k, NPC * (k + 1)))
        nodes.sort(key=lambda n: -max(len(contrib[n][0]), len(contrib[n][1])))
        slot2node.append(nodes)

    # group round counts, maxed across cores (program must be SPMD-uniform)
    r_list = []
    for g in range(GROUPS):
        r = 1
        for k in range(NCORES):
            for j in range(4):
                n = slot2node[k][4 * g + j]
                r = max(r, len(contrib[n][0]), len(contrib[n][1]))
        r_list.append(r)
    r_off = np.concatenate([[0], np.cumsum(r_list)]).astype(int)
    r_total = int(r_off[-1])

    feats2d = feats.reshape(N * C, HW)
    tab = np.concatenate([feats2d, np.zeros((C, HW), np.float32)], axis=0)

    # weight tiles (lhsT layout, replicated across the 4 row slots)
    def wtile(w, ci_lo, ci_n, co_n):
        t = np.zeros((128, 9 * co_n), np.float32)
        for rs in range(4):
            for tp in range(9):
                dy, dx = tp // 3, tp % 3
                t[32 * rs:32 * rs + ci_n, tp * co_n:(tp + 1) * co_n] = \
                    w[:, ci_lo:ci_lo + ci_n, dy, dx].T
        return t.astype(ml_dtypes.bfloat16)

    w1pn = wtile(w1, C, 2 * C, 2 * C)
    w1s = wtile(w1, 0, C, 2 * C)
    w2t = wtile(w2, 0, 2 * C, 2 * C)
    w3t = wtile(w3, 0, 2 * C, C)
    biases = np.zeros((128, 3), np.float32)

    in_maps = []
    chan = np.arange(128) % C
    for k in range(NCORES):
        gidx = np.empty((128, r_total), np.int32)
        gidx[:] = (ZROW + chan)[:, None]
        for g in range(GROUPS):
            for j in range(4):
                n = slot2node[k][4 * g + j]
                for si in range(2):
                    lst = contrib[n][si]
                    base = 32 * j + 16 * si
                    for r, m in enumerate(lst):
                        gidx[base:base + C, r_off[g] + r] = C * m + chan[:C]
        rows = np.concatenate(
            [np.arange(C * n, C * n + C) for n in slot2node[k]])
        fown = feats2d[rows]
        in_maps.append({
            "tab": tab, "gidx": gidx, "fown": np.ascontiguousarray(fown),
            "w1pn": w1pn, "w1s": w1s, "w2": w2t, "w3": w3t, "bias": biases,
        })
    return in_maps, slot2node, tuple(r_list), tuple(r_off[:-1].tolist()), r_total


def kernel(feats, edges, w1, b1, w2, b2, w3, b3):
    from concourse.bass_utils import run_bass_kernel_spmd

    in_maps, slot2node, r_list, r_off, r_total = _host_prep(
        feats, edges, w1, b1, w2, b2, w3, b3)
    with_bias = bool(np.any(np.asarray(b1)) or np.any(np.asarray(b2))
                     or np.any(np.asarray(b3)))
    assert not with_bias, "nonzero conv biases not implemented"

    key = (r_list, with_bias)
    nc = _prog_cache.get(key)
    if nc is None:
        nc = _build_program(r_list, r_off, r_total, with_bias)
        _prog_cache[key] = nc

    import os
    trace = bool(os.environ.get("KERNEL_TRACE"))
    res = run_bass_kernel_spmd(nc, in_maps, core_ids=list(range(NCORES)),
                               trace=trace)
    if trace:
        global last_results
        last_results = res

    out = np.empty((N, C, H, W), np.float32)
    for k in range(NCORES):
        yk = res.results[k]["y"]
        for i, n in enumerate(slot2node[k]):
            out[n] = yk[C * i:C * i + C].reshape(C, H, W)
    return out



# revision 9
# speedup vs baseline: 4.1445x; 1.2670x over previous
"""GNN message passing + 3x conv3x3 + leaky-relu, distributed over 8 trn2 NeuronCores.

Strategy (node-sharded, 128 nodes/core):
- Pooling (pos/neg masked bidirectional scatter-add) is done entirely by SWDGE
  indirect-gather DMAs with compute_op=add: the feats table is laid out as
  (node, channel) rows of 4 KiB; each gather round pulls 128 rows (4 nodes x
  2 signs x 16 ch) and accumulates into an SBUF tile that is ALREADY in
  conv layout (partition=channel). Zero compute-engine cost, exact fp32.
- Convs run as 9 shifted-tap matmuls over a 34x34 zero-padded grid using
  strided access-pattern views (no im2col), bf16 operands, fp32 PSUM
  accumulation. Four nodes are packed per 128-partition bundle and mapped to
  disjoint 32x32 PE sub-array rectangles via tile_position, so the 128x128
  array stays busy despite 32-channel convs.
- leaky_relu(x) = x + relu(-0.9 x): one ScalarE activation + one VectorE
  tensor_tensor add per bundle, full 128-lane ops.
"""

import numpy as np

N, C, H, W = 1024, 16, 32, 32
NCORES = 8
NPC = N // NCORES            # nodes per core
GROUPS = NPC // 4            # acc groups of 4 nodes per core
CONV_ROUNDS = NPC // 16      # 16 nodes per conv round (4 bundles)
HP = WP = H + 2
GRID = HP * WP
HW = H * W
ZROW = 16 * N                # first zero row in the gather table

_prog_cache = {}


def _make_tile_context(nc):
    """TileContext whose lowering splits multi-sem waits onto nop carriers
    (this walrus build accepts at most one sync wait per instruction) and
    whose tail drain does the same."""
    import concourse.mybir as mybir
    import concourse.tile as tile

    class _TC(tile.TileContext):
        def _lower_ordered_insts(self, ordered):
            for bb_name, insts in ordered.items():
                out = []
                for inst in insts:
                    si = inst.sync_info
                    waits = list(si.on_wait) if si is not None and si.on_wait else []
                    if len(waits) > 1:
                        for w in waits[:-1]:
                            car = mybir.InstNoOp(
                                name=self.nc.get_next_instruction_name(),
                                ins=[], outs=[])
                            car.engine = inst.engine
                            car.sync_info = mybir.SyncInfo(on_wait=[w], on_update=[])
                            self.nc.register_instruction(car, overwrite=True)
                            out.append(car)
                        inst.sync_info = mybir.SyncInfo(
                            on_wait=[waits[-1]],
                            on_update=list(si.on_update) if si.on_update else [])
                    out.append(inst)
                insts[:] = out
            return super()._lower_ordered_insts(ordered)

        def _drain_and_barrier(self, tick_clock, wait_clock):
            clock = tick_clock.global_clock
            allocated = wait_clock.sems.allocated()
            for proc, tick in enumerate(clock):
                if tick > 0 and proc in allocated:
                    n = self.nc.sync.nop(nofuse=True, hint="tailwait")
                    n.wait_op(allocated[proc], tick, "sem-ge")
            self.nc.sync.drain()
            self.nc.all_engine_barrier()
            assert self.sems is not None
            popped = self.nc._tile_sem_poison_stack.pop()
            assert popped is self._sem_poison
            self.nc.clear_and_free_semaphores(list(self.sems.allocated().values()))
            self.nc.all_engine_barrier()

    return _TC(nc)


def _build_program(r_list, r_off, r_total, with_bias, variant="full"):
    import os
    import concourse.bass as bass
    import concourse.mybir as mybir

    reps = 1
    if "x" in variant:
        variant, _, r = variant.partition("x")
        reps = int(r)
    do_gather = variant in ("full", "gather")
    do_conv = variant in ("full", "conv")

    f32 = mybir.dt.float32
    bf16 = mybir.dt.bfloat16
    i32 = mybir.dt.int32
    AF = mybir.ActivationFunctionType
    ALU = mybir.AluOpType

    nc = bass.Bass()
    tab_d = nc.dram_tensor("tab", [16 * N + 16, HW], f32, kind="ExternalInput")
    gidx_d = nc.dram_tensor("gidx", [128, r_total], i32, kind="ExternalInput")
    fown_d = nc.dram_tensor("fown", [NPC * C, HW], bf16, kind="ExternalInput")
    w1pn_d = nc.dram_tensor("w1pn", [128, 9 * 32], bf16, kind="ExternalInput")
    w1s_d = nc.dram_tensor("w1s", [128, 9 * 32], bf16, kind="ExternalInput")
    w2_d = nc.dram_tensor("w2", [128, 9 * 32], bf16, kind="ExternalInput")
    w3_d = nc.dram_tensor("w3", [128, 9 * 16], bf16, kind="ExternalInput")
    bias_d = nc.dram_tensor("bias", [128, 3], f32, kind="ExternalInput")
    y_d = nc.dram_tensor("y", [NPC * C, HW], f32, kind="ExternalOutput")

    def valid(ap_grid):
        # [p, GRID] tile AP -> [p, 32, 32] interior view of the 34x34 grid
        return ap_grid.rearrange("p (h w) -> p h w", w=WP)[:, 1:H + 1, 1:W + 1]

    def tap_view(ap_grid, base, k, dy, dx, h0):
        # rhs view for tap (dy,dx), output rows [h0, h0+16), K channels at
        # partition `base`
        g3 = ap_grid.rearrange("p (h w) -> p h w", w=WP)
        return g3[base:base + k, h0 + dy:h0 + dy + 16, dx:dx + W]

    tc = _make_tile_context(nc)
    with tc:
        with (tc.tile_pool(name="cw", bufs=1) as cw,
              tc.tile_pool(name="accp", bufs=5) as accp,
              tc.tile_pool(name="x1pnp", bufs=3) as x1pnp,
              tc.tile_pool(name="fop", bufs=3) as fop,
              tc.tile_pool(name="x1sp", bufs=3) as x1sp,
              tc.tile_pool(name="x2p", bufs=3) as x2p,
              tc.tile_pool(name="x3p", bufs=3) as x3p,
              tc.tile_pool(name="r2p", bufs=3) as r2p,
              tc.tile_pool(name="osbp", bufs=3) as osbp,
              tc.tile_pool(name="psp", bufs=4, space="PSUM") as psp):
            idx_t = cw.tile([128, r_total], i32)
            nc.sync.dma_start(out=idx_t[:], in_=gidx_d[:])
            w1pn_t = cw.tile([128, 9 * 32], bf16)
            nc.sync.dma_start(out=w1pn_t[:], in_=w1pn_d[:])
            w1s_t = cw.tile([128, 9 * 32], bf16)
            nc.sync.dma_start(out=w1s_t[:], in_=w1s_d[:])
            w2_t = cw.tile([128, 9 * 32], bf16)
            nc.sync.dma_start(out=w2_t[:], in_=w2_d[:])
            w3_t = cw.tile([128, 9 * 16], bf16)
            nc.sync.dma_start(out=w3_t[:], in_=w3_d[:])
            if with_bias:
                bias_t = cw.tile([128, 3], f32)
                nc.sync.dma_start(out=bias_t[:], in_=bias_d[:])

            memset_count = {}

            def fresh_grid(pool, name):
                t = pool.tile([128, GRID], bf16, tag=name)
                c = memset_count.get(name, 0)
                if c < 3:  # pool bufs
                    nc.vector.memset(t[:], 0.0)
                    memset_count[name] = c + 1
                return t

            def fresh_fo(pool, name):
                t = pool.tile([128, HW], bf16, tag=name)
                c = memset_count.get(name, 0)
                if c < 3:
                    nc.vector.memset(t[:], 0.0)
                    memset_count[name] = c + 1
                return t

            for rnd in [r for _ in range(reps) for r in range(CONV_ROUNDS)]:
                x1pn_tiles = []
                x1s_tiles = []
                for b in range(4):
                    g = 4 * rnd + b
                    # ---- pooling: accumulate gathers into two parallel acc
                    # tiles (halves the serial RMW chain), f32 exact
                    nsplit = 2 if r_list[g] >= 2 else 1
                    acc_t = accp.tile([128, HW], f32, tag="acc")
                    if nsplit == 2:
                        acc2_t = accp.tile([128, HW], f32, tag="acc2")
                    else:
                        acc2_t = None
                    if do_gather:
                        for r in range(r_list[g]):
                            col = r_off[g] + r
                            tgt = acc_t if (r % 2 == 0 or nsplit == 1) else acc2_t
                            nc.gpsimd.indirect_dma_start(
                                out=tgt[:], out_offset=None, in_=tab_d[:],
                                in_offset=bass.IndirectOffsetOnAxis(
                                    ap=idx_t[:, col:col + 1], axis=0),
                                compute_op=ALU.bypass if r < nsplit else ALU.add)
                    else:
                        nc.vector.memset(acc_t[:], 0.0)
                        if acc2_t is not None:
                            nc.vector.memset(acc2_t[:], 0.0)
                    if not do_conv:
                        continue
                    # ---- X1 pos/neg grid (bf16, strided valid write)
                    x1 = fresh_grid(x1pnp, "x1pn")
                    if nsplit == 2:
                        nc.vector.tensor_tensor(
                            out=valid(x1[:]),
                            in0=acc_t[:].rearrange("p (h w) -> p h w", w=W),
                            in1=acc2_t[:].rearrange("p (h w) -> p h w", w=W),
                            op=ALU.add)
                    else:
                        nc.vector.tensor_copy(
                            out=valid(x1[:]),
                            in_=acc_t[:].rearrange("p (h w) -> p h w", w=W))
                    x1pn_tiles.append(x1)
                    # ---- X1 self grid: stage feats_own (cast bf16) then copy
                    fo = fresh_fo(fop, "fo")
                    for j in range(4):
                        slot = 16 * rnd + 4 * b + j
                        nc.sync.dma_start(
                            out=fo[32 * j:32 * j + C, :],
                            in_=fown_d[C * slot:C * slot + C, :])
                    x1s = fresh_grid(x1sp, "x1s")
                    nc.vector.tensor_copy(
                        out=valid(x1s[:]),
                        in_=fo[:].rearrange("p (h w) -> p h w", w=W))
                    x1s_tiles.append(x1s)

                for b in range(4 if do_conv else 0):
                    x1, x1s = x1pn_tiles[b], x1s_tiles[b]
                    # ---- conv1: pass1 K=32 (pos+neg), pass2 K=16 (self)
                    ps1 = psp.tile([128, HW], f32, tag="ps")
                    ps1v = ps1[:].rearrange("p (h w) -> p h w", w=W)
                    for h0 in (0, 16):
                        for t in range(9):
                            dy, dx = t // 3, t % 3
                            for j in range(4):
                                cs = (j + b) % 4
                                nc.tensor.matmul(
                                    out=ps1v[32 * cs:32 * cs + 32, h0:h0 + 16, :],
                                    lhsT=w1pn_t[32 * j:32 * j + 32, t * 32:t * 32 + 32],
                                    rhs=tap_view(x1[:], 32 * j, 32, dy, dx, h0),
                                    start=(t == 0), stop=False,
                                    tile_position=(32 * j, 32 * cs))
                        for t in range(9):
                            dy, dx = t // 3, t % 3
                            for j in range(4):
                                cs = (j + b) % 4
                                nc.tensor.matmul(
                                    out=ps1v[32 * cs:32 * cs + 32, h0:h0 + 16, :],
                                    lhsT=w1s_t[32 * j:32 * j + 32, t * 32:t * 32 + 32],
                                    rhs=tap_view(x1s[:], 32 * j, 32, dy, dx, h0),
                                    start=False, stop=(t == 8),
                                    tile_position=(32 * j, 32 * cs))
                    r2a = r2p.tile([128, HW], bf16, tag="r2")
                    nc.scalar.activation(out=r2a[:], in_=ps1[:], func=AF.Relu,
                                         scale=-0.9)
                    x2 = fresh_grid(x2p, "x2")
                    nc.vector.tensor_tensor(
                        out=valid(x2[:]),
                        in0=ps1[:].rearrange("p (h w) -> p h w", w=W),
                        in1=r2a[:].rearrange("p (h w) -> p h w", w=W),
                        op=ALU.add)

                    # ---- conv2 (K=32)
                    ps2 = psp.tile([128, HW], f32, tag="ps")
                    ps2v = ps2[:].rearrange("p (h w) -> p h w", w=W)
                    for h0 in (0, 16):
                        for t in range(9):
                            dy, dx = t // 3, t % 3
                            for q in range(4):
                                cs = (q + b + 1) % 4
                                nc.tensor.matmul(
                                    out=ps2v[32 * cs:32 * cs + 32, h0:h0 + 16, :],
                                    lhsT=w2_t[32 * q:32 * q + 32, t * 32:t * 32 + 32],
                                    rhs=tap_view(x2[:], 32 * q, 32, dy, dx, h0),
                                    start=(t == 0), stop=(t == 8),
                                    tile_position=(32 * q, 32 * cs))
                    r2b = r2p.tile([128, HW], bf16, tag="r2")
                    nc.scalar.activation(out=r2b[:], in_=ps2[:], func=AF.Relu,
                                         scale=-0.9)
                    x3 = fresh_grid(x3p, "x3")
                    nc.vector.tensor_tensor(
                        out=valid(x3[:]),
                        in0=ps2[:].rearrange("p (h w) -> p h w", w=W),
                        in1=r2b[:].rearrange("p (h w) -> p h w", w=W),
                        op=ALU.add)

                    # ---- conv3 (K=32, M=16)
                    ps3 = psp.tile([128, HW], f32, tag="ps")
                    ps3v = ps3[:].rearrange("p (h w) -> p h w", w=W)
                    for h0 in (0, 16):
                        for t in range(9):
                            dy, dx = t // 3, t % 3
                            for q in range(4):
                                cs = (q + b + 2) % 4
                                nc.tensor.matmul(
                                    out=ps3v[32 * cs:32 * cs + 16, h0:h0 + 16, :],
                                    lhsT=w3_t[32 * q:32 * q + 32, t * 16:t * 16 + 16],
                                    rhs=tap_view(x3[:], 32 * q, 32, dy, dx, h0),
                                    start=(t == 0), stop=(t == 8),
                                    tile_position=(32 * q, 32 * cs))
                    r2c = r2p.tile([128, HW], bf16, tag="r2")
                    nc.scalar.activation(out=r2c[:], in_=ps3[:], func=AF.Relu,
                                         scale=-0.9)
                    osb = osbp.tile([128, HW], f32, tag="osb")
                    nc.vector.tensor_tensor(out=osb[:], in0=ps3[:], in1=r2c[:],
                                            op=ALU.add)
                    for j in range(4):
                        q1 = (j + b) % 4
                        q2 = (q1 + b + 1) % 4
                        q3 = (q2 + b + 2) % 4
                        slot = 16 * rnd + 4 * b + j
                        nc.sync.dma_start(
                            out=y_d[C * slot:C * slot + C, :],
                            in_=osb[32 * q3:32 * q3 + C, :])
    return nc


def _host_prep(feats, edges, w1, b1, w2, b2, w3, b3):
    import ml_dtypes

    feats = np.ascontiguousarray(np.asarray(feats, dtype=np.float32))
    edges = np.asarray(edges)
    w1 = np.asarray(w1, dtype=np.float32)
    w2 = np.asarray(w2, dtype=np.float32)
    w3 = np.asarray(w3, dtype=np.float32)

    # per-(node, sign) contribution lists
    contrib = [([], []) for _ in range(N)]
    for s, sg, d in edges.tolist():
        si = 0 if sg > 0 else 1
        contrib[d][si].append(s)
        contrib[s][si].append(d)

    # per-core slot ordering: sort by max degree so groups of 4 have similar
    # round counts (minimises padded gather rounds)
    slot2node = []
    for k in range(NCORES):
        nodes = list(range(NPC * k, NPC * (k + 1)))
        nodes.sort(key=lambda n: -max(len(contrib[n][0]), len(contrib[n][1])))
        slot2node.append(nodes)

    # group round counts, maxed across cores (program must be SPMD-uniform)
    r_list = []
    for g in range(GROUPS):
        r = 1
        for k in range(NCORES):
            for j in range(4):
                n = slot2node[k][4 * g + j]
                r = max(r, len(contrib[n][0]), len(contrib[n][1]))
        r_list.append(r)
    r_off = np.concatenate([[0], np.cumsum(r_list)]).astype(int)
    r_total = int(r_off[-1])

    feats2d = feats.reshape(N * C, HW)
    tab = np.concatenate([feats2d, np.zeros((C, HW), np.float32)], axis=0)

    # weight tiles (lhsT layout, replicated across the 4 row slots)
    def wtile(w, ci_lo, ci_n, co_n):
        t = np.zeros((128, 9 * co_n), np.float32)
        for rs in range(4):
            for tp in range(9):
                dy, dx = tp // 3, tp % 3
                t[32 * rs:32 * rs + ci_n, tp * co_n:(tp + 1) * co_n] = \
                    w[:, ci_lo:ci_lo + ci_n, dy, dx].T
        return t.astype(ml_dtypes.bfloat16)

    w1pn = wtile(w1, C, 2 * C, 2 * C)
    w1s = wtile(w1, 0, C, 2 * C)
    w2t = wtile(w2, 0, 2 * C, 2 * C)
    w3t = wtile(w3, 0, 2 * C, C)
    biases = np.zeros((128, 3), np.float32)

    in_maps = []
    chan = np.arange(128) % C
    for k in range(NCORES):
        gidx = np.empty((128, r_total), np.int32)
        gidx[:] = (ZROW + chan)[:, None]
        for g in range(GROUPS):
            for j in range(4):
                n = slot2node[k][4 * g + j]
                for si in range(2):
                    lst = contrib[n][si]
                    base = 32 * j + 16 * si
                    for r, m in enumerate(lst):
                        gidx[base:base + C, r_off[g] + r] = C * m + chan[:C]
        rows = np.concatenate(
            [np.arange(C * n, C * n + C) for n in slot2node[k]])
        fown = feats2d[rows].astype(ml_dtypes.bfloat16)
        in_maps.append({
            "tab": tab, "gidx": gidx, "fown": np.ascontiguousarray(fown),
            "w1pn": w1pn, "w1s": w1s, "w2": w2t, "w3": w3t, "bias": biases,
        })
    return in_maps, slot2node, tuple(r_list), tuple(r_off[:-1].tolist()), r_total


def kernel(feats, edges, w1, b1, w2, b2, w3, b3):
    from concourse.bass_utils import run_bass_kernel_spmd

    in_maps, slot2node, r_list, r_off, r_total = _host_prep(
        feats, edges, w1, b1, w2, b2, w3, b3)
    with_bias = bool(np.any(np.asarray(b1)) or np.any(np.asarray(b2))
                     or np.any(np.asarray(b3)))
    assert not with_bias, "nonzero conv biases not implemented"

    key = (r_list, with_bias)
    nc = _prog_cache.get(key)
    if nc is None:
        nc = _build_program(r_list, r_off, r_total, with_bias)
        _prog_cache[key] = nc

    import os
    trace = bool(os.environ.get("KERNEL_TRACE"))
    res = run_bass_kernel_spmd(nc, in_maps, core_ids=list(range(NCORES)),
                               trace=trace)
    if trace:
        global last_results
        last_results = res

    out = np.empty((N, C, H, W), np.float32)
    for k in range(NCORES):
        yk = res.results[k]["y"]
        for i, n in enumerate(slot2node[k]):
            out[n] = yk[C * i:C * i + C].reshape(C, H, W)
    return out



# revision 15
# speedup vs baseline: 5.4190x; 1.3075x over previous
"""GNN message passing + 3x conv3x3 + leaky-relu, distributed over 8 trn2 NeuronCores.

Strategy (node-sharded, 128 nodes/core):
- Pooling (pos/neg masked bidirectional scatter-add) is done entirely by SWDGE
  indirect-gather DMAs with compute_op=add: the feats table is laid out as
  (node, channel) rows of 4 KiB; each gather round pulls 128 rows (4 nodes x
  2 signs x 16 ch) and accumulates into an SBUF tile that is ALREADY in
  conv layout (partition=channel). Zero compute-engine cost, exact fp32.
- Convs run as 9 shifted-tap matmuls over a 34x34 zero-padded grid using
  strided access-pattern views (no im2col), bf16 operands, fp32 PSUM
  accumulation. Four nodes are packed per 128-partition bundle and mapped to
  disjoint 32x32 PE sub-array rectangles via tile_position, so the 128x128
  array stays busy despite 32-channel convs.
- leaky_relu(x) = x + relu(-0.9 x): one ScalarE activation + one VectorE
  tensor_tensor add per bundle, full 128-lane ops.
"""

import numpy as np

N, C, H, W = 1024, 16, 32, 32
NCORES = 8
NPC = N // NCORES            # nodes per core
GROUPS = NPC // 4            # acc groups of 4 nodes per core
CONV_ROUNDS = NPC // 16      # 16 nodes per conv round (4 bundles)
HP = WP = H + 2
GRID = HP * WP
HW = H * W
ZROW = 16 * N                # first zero row in the gather table

_prog_cache = {}


def _make_tile_context(nc):
    """TileContext whose lowering splits multi-sem waits onto nop carriers
    (this walrus build accepts at most one sync wait per instruction) and
    whose tail drain does the same."""
    import concourse.mybir as mybir
    import concourse.tile as tile

    class _TC(tile.TileContext):
        def _lower_ordered_insts(self, ordered):
            for bb_name, insts in ordered.items():
                out = []
                for inst in insts:
                    si = inst.sync_info
                    waits = list(si.on_wait) if si is not None and si.on_wait else []
                    if len(waits) > 1:
                        for w in waits[:-1]:
                            car = mybir.InstNoOp(
                                name=self.nc.get_next_instruction_name(),
                                ins=[], outs=[])
                            car.engine = inst.engine
                            car.sync_info = mybir.SyncInfo(on_wait=[w], on_update=[])
                            self.nc.register_instruction(car, overwrite=True)
                            out.append(car)
                        inst.sync_info = mybir.SyncInfo(
                            on_wait=[waits[-1]],
                            on_update=list(si.on_update) if si.on_update else [])
                    out.append(inst)
                insts[:] = out
            return super()._lower_ordered_insts(ordered)

        def _drain_and_barrier(self, tick_clock, wait_clock):
            clock = tick_clock.global_clock
            allocated = wait_clock.sems.allocated()
            for proc, tick in enumerate(clock):
                if tick > 0 and proc in allocated:
                    n = self.nc.sync.nop(nofuse=True, hint="tailwait")
                    n.wait_op(allocated[proc], tick, "sem-ge")
            self.nc.sync.drain()
            self.nc.all_engine_barrier()
            assert self.sems is not None
            popped = self.nc._tile_sem_poison_stack.pop()
            assert popped is self._sem_poison
            self.nc.clear_and_free_semaphores(list(self.sems.allocated().values()))
            self.nc.all_engine_barrier()

    return _TC(nc)


def _build_program(r_list, r_off, r_total, with_bias, variant="full"):
    import os
    import concourse.bass as bass
    import concourse.mybir as mybir

    reps = 1
    if "x" in variant:
        variant, _, r = variant.partition("x")
        reps = int(r)
    do_gather = variant in ("full", "gather")
    do_conv = variant in ("full", "conv")

    f32 = mybir.dt.float32
    bf16 = mybir.dt.bfloat16
    i32 = mybir.dt.int32
    AF = mybir.ActivationFunctionType
    ALU = mybir.AluOpType

    nc = bass.Bass()
    tab_d = nc.dram_tensor("tab", [16 * N + 16, HW], f32, kind="ExternalInput")
    gidx_d = nc.dram_tensor("gidx", [128, r_total], i32, kind="ExternalInput")
    fown_d = nc.dram_tensor("fown", [NPC * C, HW], bf16, kind="ExternalInput")
    w1pn_d = nc.dram_tensor("w1pn", [128, 9 * 32], bf16, kind="ExternalInput")
    w1s_d = nc.dram_tensor("w1s", [128, 9 * 32], bf16, kind="ExternalInput")
    w2_d = nc.dram_tensor("w2", [128, 9 * 32], bf16, kind="ExternalInput")
    w3_d = nc.dram_tensor("w3", [128, 9 * 16], bf16, kind="ExternalInput")
    bias_d = nc.dram_tensor("bias", [128, 3], f32, kind="ExternalInput")
    y_d = nc.dram_tensor("y", [NPC * C, HW], f32, kind="ExternalOutput")

    def valid(ap_grid):
        # [p, GRID] tile AP -> [p, 32, 32] interior view of the 34x34 grid
        return ap_grid.rearrange("p (h w) -> p h w", w=WP)[:, 1:H + 1, 1:W + 1]

    def tap_view(ap_grid, base, k, dy, dx, h0):
        # rhs view for tap (dy,dx), output rows [h0, h0+16), K channels at
        # partition `base`
        g3 = ap_grid.rearrange("p (h w) -> p h w", w=WP)
        return g3[base:base + k, h0 + dy:h0 + dy + 16, dx:dx + W]

    tc = _make_tile_context(nc)
    with tc:
        with (tc.tile_pool(name="cw", bufs=1) as cw,
              tc.tile_pool(name="accp", bufs=4) as accp,
              tc.tile_pool(name="x1pnp", bufs=4) as x1pnp,
              tc.tile_pool(name="fop", bufs=4) as fop,
              tc.tile_pool(name="x1sp", bufs=4) as x1sp,
              tc.tile_pool(name="x2p", bufs=4) as x2p,
              tc.tile_pool(name="x3p", bufs=4) as x3p,
              tc.tile_pool(name="r2p", bufs=4) as r2p,
              tc.tile_pool(name="osbp", bufs=3) as osbp,
              tc.tile_pool(name="psp", bufs=4, space="PSUM") as psp):
            idx_t = cw.tile([128, r_total], i32)
            nc.sync.dma_start(out=idx_t[:], in_=gidx_d[:])
            w1pn_t = cw.tile([128, 9 * 32], bf16)
            nc.sync.dma_start(out=w1pn_t[:], in_=w1pn_d[:])
            w1s_t = cw.tile([128, 9 * 32], bf16)
            nc.sync.dma_start(out=w1s_t[:], in_=w1s_d[:])
            w2_t = cw.tile([128, 9 * 32], bf16)
            nc.sync.dma_start(out=w2_t[:], in_=w2_d[:])
            w3_t = cw.tile([128, 9 * 16], bf16)
            nc.sync.dma_start(out=w3_t[:], in_=w3_d[:])
            if with_bias:
                bias_t = cw.tile([128, 3], f32)
                nc.sync.dma_start(out=bias_t[:], in_=bias_d[:])

            memset_count = {}

            def fresh_grid(pool, name, nbufs=4):
                t = pool.tile([128, GRID], bf16, tag=name)
                c = memset_count.get(name, 0)
                if c < nbufs:  # pool bufs
                    nc.vector.memset(t[:], 0.0)
                    memset_count[name] = c + 1
                return t

            def fresh_fo(pool, name, nbufs=4):
                t = pool.tile([128, HW], bf16, tag=name)
                c = memset_count.get(name, 0)
                if c < nbufs:
                    nc.vector.memset(t[:], 0.0)
                    memset_count[name] = c + 1
                return t

            for rnd in [r for _ in range(reps) for r in range(CONV_ROUNDS)]:
                x1pn_tiles = []
                x1s_tiles = []
                for b in range(4):
                    g = 4 * rnd + b
                    # ---- pooling: 3 parallel acc tiles hide the RMW
                    # completion latency of the gather-add chain
                    n3 = min(r_list[g], 3)
                    acc_a = accp.tile([128, HW], f32, tag="acca")
                    accs = [acc_a]
                    if n3 >= 2:
                        acc_b = accp.tile([128, HW], f32, tag="accb")
                        accs.append(acc_b)
                    if n3 >= 3:
                        acc_c = accp.tile([128, HW], f32, tag="accc")
                        accs.append(acc_c)
                    if do_gather:
                        for r in range(r_list[g]):
                            col = r_off[g] + r
                            nc.gpsimd.indirect_dma_start(
                                out=accs[r % n3][:], out_offset=None,
                                in_=tab_d[:],
                                in_offset=bass.IndirectOffsetOnAxis(
                                    ap=idx_t[:, col:col + 1], axis=0),
                                compute_op=ALU.bypass if r < n3 else ALU.add)
                    else:
                        for a in accs:
                            nc.vector.memset(a[:], 0.0)
                    if not do_conv:
                        continue
                    # ---- X1 pos/neg grid (bf16, strided valid write)
                    x1 = fresh_grid(x1pnp, "x1pn")
                    if n3 == 1:
                        nc.vector.tensor_copy(
                            out=valid(x1[:]),
                            in_=accs[0][:].rearrange("p (h w) -> p h w", w=W))
                    else:
                        nc.vector.tensor_tensor(
                            out=valid(x1[:]),
                            in0=accs[0][:].rearrange("p (h w) -> p h w", w=W),
                            in1=accs[1][:].rearrange("p (h w) -> p h w", w=W),
                            op=ALU.add)
                        if n3 >= 3:
                            nc.vector.tensor_tensor(
                                out=valid(x1[:]),
                                in0=valid(x1[:]),
                                in1=accs[2][:].rearrange("p (h w) -> p h w",
                                                         w=W),
                                op=ALU.add)
                    x1pn_tiles.append(x1)
                    # ---- X1 self grid: stage feats_own (bf16) then copy
                    fo = fresh_fo(fop, "fo")
                    for j in range(4):
                        slot = 16 * rnd + 4 * b + j
                        nc.sync.dma_start(
                            out=fo[32 * j:32 * j + C, :],
                            in_=fown_d[C * slot:C * slot + C, :])
                    x1s = fresh_grid(x1sp, "x1s")
                    nc.vector.tensor_copy(
                        out=valid(x1s[:]),
                        in_=fo[:].rearrange("p (h w) -> p h w", w=W))
                    x1s_tiles.append(x1s)

                # ---- convs: waves of 2 bundles -> 8-way PE tile packing;
                # cross-stage overlap via the 4-slot PSUM pool
                for wv in range(2 if do_conv else 0):
                    bs = (2 * wv, 2 * wv + 1)
                    ps1 = {}
                    for b in bs:
                        ps1[b] = psp.tile([128, HW], f32, tag="ps", name="ps1")
                    for h0 in (0, 16):
                        for t in range(9):
                            dy, dx = t // 3, t % 3
                            for b in bs:
                                pv = ps1[b][:].rearrange("p (h w) -> p h w",
                                                         w=W)
                                for j in range(4):
                                    cs = (j + b) % 4
                                    nc.tensor.matmul(
                                        out=pv[32 * cs:32 * cs + 32,
                                               h0:h0 + 16, :],
                                        lhsT=w1pn_t[32 * j:32 * j + 32,
                                                    t * 32:t * 32 + 32],
                                        rhs=tap_view(x1pn_tiles[b][:], 32 * j,
                                                     32, dy, dx, h0),
                                        start=(t == 0), stop=False,
                                        tile_position=(32 * j, 32 * cs))
                        for t in range(9):
                            dy, dx = t // 3, t % 3
                            for b in bs:
                                pv = ps1[b][:].rearrange("p (h w) -> p h w",
                                                         w=W)
                                for j in range(4):
                                    cs = (j + b) % 4
                                    nc.tensor.matmul(
                                        out=pv[32 * cs:32 * cs + 32,
                                               h0:h0 + 16, :],
                                        lhsT=w1s_t[32 * j:32 * j + 32,
                                                   t * 32:t * 32 + 32],
                                        rhs=tap_view(x1s_tiles[b][:], 32 * j,
                                                     32, dy, dx, h0),
                                        start=False, stop=(t == 8),
                                        tile_position=(32 * j, 32 * cs))
                    x2t = {}
                    for b in bs:
                        r2a = r2p.tile([128, HW], bf16, tag="r2", name="r2a")
                        nc.scalar.activation(out=r2a[:], in_=ps1[b][:],
                                             func=AF.Relu, scale=-0.9)
                        x2t[b] = fresh_grid(x2p, "x2")
                        nc.vector.tensor_tensor(
                            out=valid(x2t[b][:]),
                            in0=ps1[b][:].rearrange("p (h w) -> p h w", w=W),
                            in1=r2a[:].rearrange("p (h w) -> p h w", w=W),
                            op=ALU.add)

                    # ---- conv2 (K=32)
                    ps2 = {}
                    for b in bs:
                        ps2[b] = psp.tile([128, HW], f32, tag="ps", name="ps2")
                    for h0 in (0, 16):
                        for t in range(9):
                            dy, dx = t // 3, t % 3
                            for b in bs:
                                pv = ps2[b][:].rearrange("p (h w) -> p h w",
                                                         w=W)
                                for q in range(4):
                                    cs = (q + b + 1) % 4
                                    nc.tensor.matmul(
                                        out=pv[32 * cs:32 * cs + 32,
                                               h0:h0 + 16, :],
                                        lhsT=w2_t[32 * q:32 * q + 32,
                                                  t * 32:t * 32 + 32],
                                        rhs=tap_view(x2t[b][:], 32 * q, 32,
                                                     dy, dx, h0),
                                        start=(t == 0), stop=(t == 8),
                                        tile_position=(32 * q, 32 * cs))
                    x3t = {}
                    for b in bs:
                        r2b = r2p.tile([128, HW], bf16, tag="r2", name="r2b")
                        nc.scalar.activation(out=r2b[:], in_=ps2[b][:],
                                             func=AF.Relu, scale=-0.9)
                        x3t[b] = fresh_grid(x3p, "x3")
                        nc.vector.tensor_tensor(
                            out=valid(x3t[b][:]),
                            in0=ps2[b][:].rearrange("p (h w) -> p h w", w=W),
                            in1=r2b[:].rearrange("p (h w) -> p h w", w=W),
                            op=ALU.add)

                    # ---- conv3 (K=32, M=16)
                    ps3 = {}
                    for b in bs:
                        ps3[b] = psp.tile([128, HW], f32, tag="ps", name="ps3")
                    for h0 in (0, 16):
                        for t in range(9):
                            dy, dx = t // 3, t % 3
                            for b in bs:
                                pv = ps3[b][:].rearrange("p (h w) -> p h w",
                                                         w=W)
                                for q in range(4):
                                    cs = (q + b + 2) % 4
                                    nc.tensor.matmul(
                                        out=pv[32 * cs:32 * cs + 16,
                                               h0:h0 + 16, :],
                                        lhsT=w3_t[32 * q:32 * q + 32,
                                                  t * 16:t * 16 + 16],
                                        rhs=tap_view(x3t[b][:], 32 * q, 32,
                                                     dy, dx, h0),
                                        start=(t == 0), stop=(t == 8),
                                        tile_position=(32 * q, 32 * cs))
                    for b in bs:
                        r2c = r2p.tile([128, HW], bf16, tag="r2", name="r2c")
                        nc.scalar.activation(out=r2c[:], in_=ps3[b][:],
                                             func=AF.Relu, scale=-0.9)
                        osb = osbp.tile([128, HW], f32, tag="osb")
                        nc.vector.tensor_tensor(out=osb[:], in0=ps3[b][:],
                                                in1=r2c[:], op=ALU.add)
                        for j in range(4):
                            q3 = (j + 3 * b + 3) % 4
                            slot = 16 * rnd + 4 * b + j
                            nc.sync.dma_start(
                                out=y_d[C * slot:C * slot + C, :],
                                in_=osb[32 * q3:32 * q3 + C, :])
    return nc


def _host_prep(feats, edges, w1, b1, w2, b2, w3, b3):
    import ml_dtypes

    feats = np.ascontiguousarray(np.asarray(feats, dtype=np.float32))
    edges = np.asarray(edges)
    w1 = np.asarray(w1, dtype=np.float32)
    w2 = np.asarray(w2, dtype=np.float32)
    w3 = np.asarray(w3, dtype=np.float32)

    # per-(node, sign) contribution lists
    contrib = [([], []) for _ in range(N)]
    for s, sg, d in edges.tolist():
        si = 0 if sg > 0 else 1
        contrib[d][si].append(s)
        contrib[s][si].append(d)

    # per-core slot ordering: sort by max degree (ascending) so groups of 4
    # have similar round counts (minimises padded gather rounds) and the
    # first groups are cheap (fast pipeline startup)
    slot2node = []
    for k in range(NCORES):
        nodes = list(range(NPC * k, NPC * (k + 1)))
        nodes.sort(key=lambda n: max(len(contrib[n][0]), len(contrib[n][1])))
        slot2node.append(nodes)

    # group round counts, maxed across cores (program must be SPMD-uniform)
    r_list = []
    for g in range(GROUPS):
        r = 1
        for k in range(NCORES):
            for j in range(4):
                n = slot2node[k][4 * g + j]
                r = max(r, len(contrib[n][0]), len(contrib[n][1]))
        r_list.append(r)
    r_off = np.concatenate([[0], np.cumsum(r_list)]).astype(int)
    r_total = int(r_off[-1])

    feats2d = feats.reshape(N * C, HW)
    tab = np.concatenate([feats2d, np.zeros((C, HW), np.float32)], axis=0)

    # weight tiles (lhsT layout, replicated across the 4 row slots)
    def wtile(w, ci_lo, ci_n, co_n):
        t = np.zeros((128, 9 * co_n), np.float32)
        for rs in range(4):
            for tp in range(9):
                dy, dx = tp // 3, tp % 3
                t[32 * rs:32 * rs + ci_n, tp * co_n:(tp + 1) * co_n] = \
                    w[:, ci_lo:ci_lo + ci_n, dy, dx].T
        return t.astype(ml_dtypes.bfloat16)

    w1pn = wtile(w1, C, 2 * C, 2 * C)
    w1s = wtile(w1, 0, C, 2 * C)
    w2t = wtile(w2, 0, 2 * C, 2 * C)
    w3t = wtile(w3, 0, 2 * C, C)
    biases = np.zeros((128, 3), np.float32)

    in_maps = []
    chan = np.arange(128) % C
    for k in range(NCORES):
        gidx = np.empty((128, r_total), np.int32)
        gidx[:] = (ZROW + chan)[:, None]
        for g in range(GROUPS):
            for j in range(4):
                n = slot2node[k][4 * g + j]
                for si in range(2):
                    lst = contrib[n][si]
                    base = 32 * j + 16 * si
                    for r, m in enumerate(lst):
                        gidx[base:base + C, r_off[g] + r] = C * m + chan[:C]
        rows = np.concatenate(
            [np.arange(C * n, C * n + C) for n in slot2node[k]])
        fown = feats2d[rows].astype(ml_dtypes.bfloat16)
        in_maps.append({
            "tab": tab, "gidx": gidx, "fown": np.ascontiguousarray(fown),
            "w1pn": w1pn, "w1s": w1s, "w2": w2t, "w3": w3t, "bias": biases,
        })
    return in_maps, slot2node, tuple(r_list), tuple(r_off[:-1].tolist()), r_total


def kernel(feats, edges, w1, b1, w2, b2, w3, b3):
    from concourse.bass_utils import run_bass_kernel_spmd

    in_maps, slot2node, r_list, r_off, r_total = _host_prep(
        feats, edges, w1, b1, w2, b2, w3, b3)
    with_bias = bool(np.any(np.asarray(b1)) or np.any(np.asarray(b2))
                     or np.any(np.asarray(b3)))
    assert not with_bias, "nonzero conv biases not implemented"

    key = (r_list, with_bias)
    nc = _prog_cache.get(key)
    if nc is None:
        nc = _build_program(r_list, r_off, r_total, with_bias)
        _prog_cache[key] = nc

    import os
    trace = bool(os.environ.get("KERNEL_TRACE"))
    res = run_bass_kernel_spmd(nc, in_maps, core_ids=list(range(NCORES)),
                               trace=trace)
    if trace:
        global last_results
        last_results = res

    out = np.empty((N, C, H, W), np.float32)
    for k in range(NCORES):
        yk = res.results[k]["y"]
        for i, n in enumerate(slot2node[k]):
            out[n] = yk[C * i:C * i + C].reshape(C, H, W)
    return out



# revision 16
# speedup vs baseline: 6.4239x; 1.1854x over previous
"""GNN message passing + 3x conv3x3 + leaky-relu, distributed over 8 trn2 NeuronCores.

Strategy (node-sharded, 128 nodes/core):
- Pooling (pos/neg masked bidirectional scatter-add) is done entirely by SWDGE
  indirect-gather DMAs with compute_op=add: the feats table is laid out as
  (node, channel) rows of 4 KiB; each gather round pulls 128 rows (4 nodes x
  2 signs x 16 ch) and accumulates into an SBUF tile that is ALREADY in
  conv layout (partition=channel). Zero compute-engine cost, exact fp32.
- Convs run as 9 shifted-tap matmuls over a 34x34 zero-padded grid using
  strided access-pattern views (no im2col), bf16 operands, fp32 PSUM
  accumulation. Four nodes are packed per 128-partition bundle and mapped to
  disjoint 32x32 PE sub-array rectangles via tile_position, so the 128x128
  array stays busy despite 32-channel convs.
- leaky_relu(x) = x + relu(-0.9 x): one ScalarE activation + one VectorE
  tensor_tensor add per bundle, full 128-lane ops.
"""

import numpy as np

N, C, H, W = 1024, 16, 32, 32
NCORES = 8
NPC = N // NCORES            # nodes per core
GROUPS = NPC // 4            # acc groups of 4 nodes per core
CONV_ROUNDS = NPC // 16      # 16 nodes per conv round (4 bundles)
HP = WP = H + 2
GRID = HP * WP
HW = H * W
ZROW = 16 * N                # first zero row in the gather table

_prog_cache = {}


def _make_tile_context(nc):
    """TileContext whose lowering splits multi-sem waits onto nop carriers
    (this walrus build accepts at most one sync wait per instruction) and
    whose tail drain does the same."""
    import concourse.mybir as mybir
    import concourse.tile as tile

    class _TC(tile.TileContext):
        def _lower_ordered_insts(self, ordered):
            for bb_name, insts in ordered.items():
                out = []
                for inst in insts:
                    si = inst.sync_info
                    waits = list(si.on_wait) if si is not None and si.on_wait else []
                    if len(waits) > 1:
                        for w in waits[:-1]:
                            car = mybir.InstNoOp(
                                name=self.nc.get_next_instruction_name(),
                                ins=[], outs=[])
                            car.engine = inst.engine
                            car.sync_info = mybir.SyncInfo(on_wait=[w], on_update=[])
                            self.nc.register_instruction(car, overwrite=True)
                            out.append(car)
                        inst.sync_info = mybir.SyncInfo(
                            on_wait=[waits[-1]],
                            on_update=list(si.on_update) if si.on_update else [])
                    out.append(inst)
                insts[:] = out
            return super()._lower_ordered_insts(ordered)

        def _drain_and_barrier(self, tick_clock, wait_clock):
            clock = tick_clock.global_clock
            allocated = wait_clock.sems.allocated()
            for proc, tick in enumerate(clock):
                if tick > 0 and proc in allocated:
                    n = self.nc.sync.nop(nofuse=True, hint="tailwait")
                    n.wait_op(allocated[proc], tick, "sem-ge")
            self.nc.sync.drain()
            self.nc.all_engine_barrier()
            assert self.sems is not None
            popped = self.nc._tile_sem_poison_stack.pop()
            assert popped is self._sem_poison
            self.nc.clear_and_free_semaphores(list(self.sems.allocated().values()))
            self.nc.all_engine_barrier()

    return _TC(nc)


def _build_program(r_list, r_off, r_total, with_bias, variant="full"):
    import os
    import concourse.bass as bass
    import concourse.mybir as mybir

    reps = 1
    if "x" in variant:
        variant, _, r = variant.partition("x")
        reps = int(r)
    do_gather = variant in ("full", "gather")
    do_conv = variant in ("full", "conv")

    f32 = mybir.dt.float32
    bf16 = mybir.dt.bfloat16
    i32 = mybir.dt.int32
    AF = mybir.ActivationFunctionType
    ALU = mybir.AluOpType

    nc = bass.Bass()
    tab_d = nc.dram_tensor("tab", [16 * N + 16, HW], f32, kind="ExternalInput")
    gidx_d = nc.dram_tensor("gidx", [128, r_total], i32, kind="ExternalInput")
    fown_d = nc.dram_tensor("fown", [NPC * C, HW], bf16, kind="ExternalInput")
    w1pn_d = nc.dram_tensor("w1pn", [128, 9 * 32], bf16, kind="ExternalInput")
    w1s_d = nc.dram_tensor("w1s", [128, 9 * 32], bf16, kind="ExternalInput")
    w2_d = nc.dram_tensor("w2", [128, 9 * 32], bf16, kind="ExternalInput")
    w3_d = nc.dram_tensor("w3", [128, 9 * 16], bf16, kind="ExternalInput")
    bias_d = nc.dram_tensor("bias", [128, 3], f32, kind="ExternalInput")
    y_d = nc.dram_tensor("y", [NPC * C, HW], f32, kind="ExternalOutput")

    def valid(ap_grid):
        # [p, GRID] tile AP -> [p, 32, 32] interior view of the 34x34 grid
        return ap_grid.rearrange("p (h w) -> p h w", w=WP)[:, 1:H + 1, 1:W + 1]

    def tap_view(ap_grid, base, k, dy, dx, h0):
        # rhs view for tap (dy,dx), output rows [h0, h0+16), K channels at
        # partition `base`
        g3 = ap_grid.rearrange("p (h w) -> p h w", w=WP)
        return g3[base:base + k, h0 + dy:h0 + dy + 16, dx:dx + W]

    tc = _make_tile_context(nc)
    with tc:
        with (tc.tile_pool(name="cw", bufs=1) as cw,
              tc.tile_pool(name="accp", bufs=6) as accp,
              tc.tile_pool(name="x1pnp", bufs=4) as x1pnp,
              tc.tile_pool(name="fop", bufs=4) as fop,
              tc.tile_pool(name="x1sp", bufs=4) as x1sp,
              tc.tile_pool(name="x2p", bufs=4) as x2p,
              tc.tile_pool(name="x3p", bufs=4) as x3p,
              tc.tile_pool(name="r2p", bufs=4) as r2p,
              tc.tile_pool(name="osbp", bufs=3) as osbp,
              tc.tile_pool(name="psp", bufs=4, space="PSUM") as psp):
            idx_t = cw.tile([128, r_total], i32)
            nc.sync.dma_start(out=idx_t[:], in_=gidx_d[:])
            w1pn_t = cw.tile([128, 9 * 32], bf16)
            nc.sync.dma_start(out=w1pn_t[:], in_=w1pn_d[:])
            w1s_t = cw.tile([128, 9 * 32], bf16)
            nc.sync.dma_start(out=w1s_t[:], in_=w1s_d[:])
            w2_t = cw.tile([128, 9 * 32], bf16)
            nc.sync.dma_start(out=w2_t[:], in_=w2_d[:])
            w3_t = cw.tile([128, 9 * 16], bf16)
            nc.sync.dma_start(out=w3_t[:], in_=w3_d[:])
            if with_bias:
                bias_t = cw.tile([128, 3], f32)
                nc.sync.dma_start(out=bias_t[:], in_=bias_d[:])

            memset_count = {}

            def fresh_grid(pool, name, nbufs=4):
                t = pool.tile([128, GRID], bf16, tag=name)
                c = memset_count.get(name, 0)
                if c < nbufs:  # pool bufs
                    nc.scalar.memzero(t[:])
                    memset_count[name] = c + 1
                return t

            def fresh_fo(pool, name, nbufs=4):
                t = pool.tile([128, HW], bf16, tag=name)
                c = memset_count.get(name, 0)
                if c < nbufs:
                    nc.scalar.memzero(t[:])
                    memset_count[name] = c + 1
                return t

            for rnd in [r for _ in range(reps) for r in range(CONV_ROUNDS)]:
                x1pn_tiles = []
                x1s_tiles = []
                for b in range(4):
                    g = 4 * rnd + b
                    # ---- pooling: 3 parallel acc tiles hide the RMW
                    # completion latency of the gather-add chain
                    n3 = min(r_list[g], 3)
                    acc_a = accp.tile([128, HW], f32, tag="acca")
                    accs = [acc_a]
                    if n3 >= 2:
                        acc_b = accp.tile([128, HW], f32, tag="accb")
                        accs.append(acc_b)
                    if n3 >= 3:
                        acc_c = accp.tile([128, HW], f32, tag="accc")
                        accs.append(acc_c)
                    if do_gather:
                        for r in range(r_list[g]):
                            col = r_off[g] + r
                            nc.gpsimd.indirect_dma_start(
                                out=accs[r % n3][:], out_offset=None,
                                in_=tab_d[:],
                                in_offset=bass.IndirectOffsetOnAxis(
                                    ap=idx_t[:, col:col + 1], axis=0),
                                compute_op=ALU.bypass if r < n3 else ALU.add)
                    else:
                        for a in accs:
                            nc.vector.memset(a[:], 0.0)
                    if not do_conv:
                        continue
                    # ---- X1 pos/neg grid (bf16, strided valid write)
                    x1 = fresh_grid(x1pnp, "x1pn")
                    if n3 == 1:
                        nc.vector.tensor_copy(
                            out=valid(x1[:]),
                            in_=accs[0][:].rearrange("p (h w) -> p h w", w=W))
                    else:
                        nc.vector.tensor_tensor(
                            out=valid(x1[:]),
                            in0=accs[0][:].rearrange("p (h w) -> p h w", w=W),
                            in1=accs[1][:].rearrange("p (h w) -> p h w", w=W),
                            op=ALU.add)
                        if n3 >= 3:
                            nc.vector.tensor_tensor(
                                out=valid(x1[:]),
                                in0=valid(x1[:]),
                                in1=accs[2][:].rearrange("p (h w) -> p h w",
                                                         w=W),
                                op=ALU.add)
                    x1pn_tiles.append(x1)
                    # ---- X1 self grid: stage feats_own (bf16) then copy
                    fo = fresh_fo(fop, "fo")
                    for j in range(4):
                        slot = 16 * rnd + 4 * b + j
                        nc.sync.dma_start(
                            out=fo[32 * j:32 * j + C, :],
                            in_=fown_d[C * slot:C * slot + C, :])
                    x1s = fresh_grid(x1sp, "x1s")
                    nc.vector.tensor_copy(
                        out=valid(x1s[:]),
                        in_=fo[:].rearrange("p (h w) -> p h w", w=W))
                    x1s_tiles.append(x1s)

                # ---- convs: waves of 2 bundles -> 8-way PE tile packing;
                # cross-stage overlap via the 4-slot PSUM pool
                for wv in range(2 if do_conv else 0):
                    bs = (2 * wv, 2 * wv + 1)
                    ps1 = {}
                    for b in bs:
                        ps1[b] = psp.tile([128, HW], f32, tag="ps", name="ps1")
                    for h0 in (0, 16):
                        for t in range(9):
                            dy, dx = t // 3, t % 3
                            for b in bs:
                                pv = ps1[b][:].rearrange("p (h w) -> p h w",
                                                         w=W)
                                for j in range(4):
                                    cs = (j + b) % 4
                                    nc.tensor.matmul(
                                        out=pv[32 * cs:32 * cs + 32,
                                               h0:h0 + 16, :],
                                        lhsT=w1s_t[32 * j:32 * j + 32,
                                                   t * 32:t * 32 + 32],
                                        rhs=tap_view(x1s_tiles[b][:], 32 * j,
                                                     32, dy, dx, h0),
                                        start=(t == 0), stop=False,
                                        tile_position=(32 * j, 32 * cs))
                        for t in range(9):
                            dy, dx = t // 3, t % 3
                            for b in bs:
                                pv = ps1[b][:].rearrange("p (h w) -> p h w",
                                                         w=W)
                                for j in range(4):
                                    cs = (j + b) % 4
                                    nc.tensor.matmul(
                                        out=pv[32 * cs:32 * cs + 32,
                                               h0:h0 + 16, :],
                                        lhsT=w1pn_t[32 * j:32 * j + 32,
                                                    t * 32:t * 32 + 32],
                                        rhs=tap_view(x1pn_tiles[b][:], 32 * j,
                                                     32, dy, dx, h0),
                                        start=False, stop=(t == 8),
                                        tile_position=(32 * j, 32 * cs))
                    x2t = {}
                    for b in bs:
                        r2a = r2p.tile([128, HW], bf16, tag="r2", name="r2a")
                        nc.scalar.activation(out=r2a[:], in_=ps1[b][:],
                                             func=AF.Relu, scale=-0.9)
                        x2t[b] = fresh_grid(x2p, "x2")
                        nc.vector.tensor_tensor(
                            out=valid(x2t[b][:]),
                            in0=ps1[b][:].rearrange("p (h w) -> p h w", w=W),
                            in1=r2a[:].rearrange("p (h w) -> p h w", w=W),
                            op=ALU.add)

                    # ---- conv2 (K=32)
                    ps2 = {}
                    for b in bs:
                        ps2[b] = psp.tile([128, HW], f32, tag="ps", name="ps2")
                    for h0 in (0, 16):
                        for t in range(9):
                            dy, dx = t // 3, t % 3
                            for b in bs:
                                pv = ps2[b][:].rearrange("p (h w) -> p h w",
                                                         w=W)
                                for q in range(4):
                                    cs = (q + b + 1) % 4
                                    nc.tensor.matmul(
                                        out=pv[32 * cs:32 * cs + 32,
                                               h0:h0 + 16, :],
                                        lhsT=w2_t[32 * q:32 * q + 32,
                                                  t * 32:t * 32 + 32],
                                        rhs=tap_view(x2t[b][:], 32 * q, 32,
                                                     dy, dx, h0),
                                        start=(t == 0), stop=(t == 8),
                                        tile_position=(32 * q, 32 * cs))
                    x3t = {}
                    for b in bs:
                        r2b = r2p.tile([128, HW], bf16, tag="r2", name="r2b")
                        nc.scalar.activation(out=r2b[:], in_=ps2[b][:],
                                             func=AF.Relu, scale=-0.9)
                        x3t[b] = fresh_grid(x3p, "x3")
                        nc.vector.tensor_tensor(
                            out=valid(x3t[b][:]),
                            in0=ps2[b][:].rearrange("p (h w) -> p h w", w=W),
                            in1=r2b[:].rearrange("p (h w) -> p h w", w=W),
                            op=ALU.add)

                    # ---- conv3 (K=32, M=16)
                    ps3 = {}
                    for b in bs:
                        ps3[b] = psp.tile([128, HW], f32, tag="ps", name="ps3")
                    for h0 in (0, 16):
                        for t in range(9):
                            dy, dx = t // 3, t % 3
                            for b in bs:
                                pv = ps3[b][:].rearrange("p (h w) -> p h w",
                                                         w=W)
                                for q in range(4):
                                    cs = (q + b + 2) % 4
                                    nc.tensor.matmul(
                                        out=pv[32 * cs:32 * cs + 16,
                                               h0:h0 + 16, :],
                                        lhsT=w3_t[32 * q:32 * q + 32,
                                                  t * 16:t * 16 + 16],
                                        rhs=tap_view(x3t[b][:], 32 * q, 32,
                                                     dy, dx, h0),
                                        start=(t == 0), stop=(t == 8),
                                        tile_position=(32 * q, 32 * cs))
                    for b in bs:
                        r2c = r2p.tile([128, HW], bf16, tag="r2", name="r2c")
                        nc.scalar.activation(out=r2c[:], in_=ps3[b][:],
                                             func=AF.Relu, scale=-0.9)
                        osb = osbp.tile([128, HW], f32, tag="osb")
                        nc.vector.tensor_tensor(out=osb[:], in0=ps3[b][:],
                                                in1=r2c[:], op=ALU.add)
                        for j in range(4):
                            q3 = (j + 3 * b + 3) % 4
                            slot = 16 * rnd + 4 * b + j
                            nc.sync.dma_start(
                                out=y_d[C * slot:C * slot + C, :],
                                in_=osb[32 * q3:32 * q3 + C, :])
    return nc


def _host_prep(feats, edges, w1, b1, w2, b2, w3, b3):
    import ml_dtypes

    feats = np.ascontiguousarray(np.asarray(feats, dtype=np.float32))
    edges = np.asarray(edges)
    w1 = np.asarray(w1, dtype=np.float32)
    w2 = np.asarray(w2, dtype=np.float32)
    w3 = np.asarray(w3, dtype=np.float32)

    # per-(node, sign) contribution lists
    contrib = [([], []) for _ in range(N)]
    for s, sg, d in edges.tolist():
        si = 0 if sg > 0 else 1
        contrib[d][si].append(s)
        contrib[s][si].append(d)

    # global ascending degree sort, dealt round-robin across cores: every
    # core's group g holds nodes from the same degree band, minimising the
    # cross-core max that pads gather rounds; first groups are cheap so the
    # pipeline starts fast
    order = sorted(range(N),
                   key=lambda n: max(len(contrib[n][0]), len(contrib[n][1])))
    slot2node = [[] for _ in range(NCORES)]
    for g in range(GROUPS):
        band = order[4 * NCORES * g:4 * NCORES * (g + 1)]
        for k in range(NCORES):
            slot2node[k].extend(band[4 * k:4 * k + 4])

    # group round counts, maxed across cores (program must be SPMD-uniform)
    r_list = []
    for g in range(GROUPS):
        r = 1
        for k in range(NCORES):
            for j in range(4):
                n = slot2node[k][4 * g + j]
                r = max(r, len(contrib[n][0]), len(contrib[n][1]))
        r_list.append(r)
    r_off = np.concatenate([[0], np.cumsum(r_list)]).astype(int)
    r_total = int(r_off[-1])

    feats2d = feats.reshape(N * C, HW)
    tab = np.concatenate([feats2d, np.zeros((C, HW), np.float32)], axis=0)

    # weight tiles (lhsT layout, replicated across the 4 row slots)
    def wtile(w, ci_lo, ci_n, co_n):
        t = np.zeros((128, 9 * co_n), np.float32)
        for rs in range(4):
            for tp in range(9):
                dy, dx = tp // 3, tp % 3
                t[32 * rs:32 * rs + ci_n, tp * co_n:(tp + 1) * co_n] = \
                    w[:, ci_lo:ci_lo + ci_n, dy, dx].T
        return t.astype(ml_dtypes.bfloat16)

    w1pn = wtile(w1, C, 2 * C, 2 * C)
    w1s = wtile(w1, 0, C, 2 * C)
    w2t = wtile(w2, 0, 2 * C, 2 * C)
    w3t = wtile(w3, 0, 2 * C, C)
    biases = np.zeros((128, 3), np.float32)

    in_maps = []
    chan = np.arange(128) % C
    for k in range(NCORES):
        gidx = np.empty((128, r_total), np.int32)
        gidx[:] = (ZROW + chan)[:, None]
        for g in range(GROUPS):
            for j in range(4):
                n = slot2node[k][4 * g + j]
                for si in range(2):
                    lst = contrib[n][si]
                    base = 32 * j + 16 * si
                    for r, m in enumerate(lst):
                        gidx[base:base + C, r_off[g] + r] = C * m + chan[:C]
        rows = np.concatenate(
            [np.arange(C * n, C * n + C) for n in slot2node[k]])
        fown = feats2d[rows].astype(ml_dtypes.bfloat16)
        in_maps.append({
            "tab": tab, "gidx": gidx, "fown": np.ascontiguousarray(fown),
            "w1pn": w1pn, "w1s": w1s, "w2": w2t, "w3": w3t, "bias": biases,
        })
    return in_maps, slot2node, tuple(r_list), tuple(r_off[:-1].tolist()), r_total


def kernel(feats, edges, w1, b1, w2, b2, w3, b3):
    from concourse.bass_utils import run_bass_kernel_spmd

    in_maps, slot2node, r_list, r_off, r_total = _host_prep(
        feats, edges, w1, b1, w2, b2, w3, b3)
    with_bias = bool(np.any(np.asarray(b1)) or np.any(np.asarray(b2))
                     or np.any(np.asarray(b3)))
    assert not with_bias, "nonzero conv biases not implemented"

    key = (r_list, with_bias)
    nc = _prog_cache.get(key)
    if nc is None:
        nc = _build_program(r_list, r_off, r_total, with_bias)
        _prog_cache[key] = nc

    import os
    trace = bool(os.environ.get("KERNEL_TRACE"))
    res = run_bass_kernel_spmd(nc, in_maps, core_ids=list(range(NCORES)),
                               trace=trace)
    if trace:
        global last_results
        last_results = res

    out = np.empty((N, C, H, W), np.float32)
    for k in range(NCORES):
        yk = res.results[k]["y"]
        for i, n in enumerate(slot2node[k]):
            out[n] = yk[C * i:C * i + C].reshape(C, H, W)
    return out

